# revision 15
# baseline (speedup 1.0000x reference)
"""Trainium2 Bass kernel for nn_CEmbedder_L: 36 independent scalar-input MLPs.

Reference computation (fp32):
    h   = leaky_relu(x[:, :, None] * W1[None] + b1[None])   # [B, 36, 512]
    out = einsum('bih,ihd->bid', h, W2) + b2[None]          # [B, 36, 1024]

Sharding across 8 NeuronCores: 2 batch halves x 4 branch groups.
Core c handles batch rows [1024*(c//4), 1024*(c//4+1)) and branches
[9*(c%4), 9*(c%4+1)). Every core runs the same SPMD program on its own
shard (uniform shapes), so one Bass program serves all 8 cores.

Per-core dataflow (B=1024 batch rows, NB=9 branches):
  - x arrives transposed per branch group: x_t[i] = x[:, branch i] (one
    row of B values). A DMA with a partition-broadcast source replicates
    it to a [128, B] SBUF tile.
  - fc1 per 128-wide hid chunk k is one ScalarE activation:
    h^T[k] = Lrelu(x_bcast * W1[i,k-chunk] + b1[i,k-chunk]) with
    per-partition scale/bias - h^T laid out [hid, batch] as the matmul
    needs. Output dtype float32r (rounded fp32, full-rate on the PE,
    ~1.6e-4 relative rounding vs 4x-slower exact fp32).
  - fc2 is a [1024, 512] @ [512, 1024] GEMM per branch on TensorE with
    float32r operands: out[m, n] += h^T[k][:, m-chunk].T @ W2[i,k][:, n]
    accumulated over k in fp32 PSUM; VectorE adds b2 (DMA-broadcast)
    while evacuating PSUM -> SBUF; DMA writes [128, 1024] rows out.
"""

import sys

if "/opt/trn_rl_repo" not in sys.path:
    sys.path.insert(0, "/opt/trn_rl_repo")

import numpy as np

import concourse.bass as bass
import concourse.mybir as mybir
import concourse.tile as tile
from concourse.bass_utils import run_bass_kernel_spmd

B_FULL = 2048
IN_DIM = 36
HID = 512
EMB = 1024
NEG_SLOPE = 0.01

N_CORES = 8
BATCH_SPLIT = 2
BRANCH_SPLIT = 4
B = B_FULL // BATCH_SPLIT          # 1024 batch rows per core
NB = IN_DIM // BRANCH_SPLIT        # 9 branches per core
KC = HID // 128                    # 4 contraction chunks of 128
MC = B // 128                      # 8 batch chunks of 128
P = 128

F32 = mybir.dt.float32
F32R = mybir.dt.float32r

_compiled = None


def _split_excess_waits(nc, max_waits=1):
    """The walrus build in this container rejects instructions carrying
    more than one sync wait ("Too many sync wait commands", setupSyncWait)
    instead of auto-splitting them. Move excess waits onto same-engine
    NoOp carriers placed immediately before the instruction -
    engine-serial execution preserves wait-then-proceed semantics."""
    import bass_rust
    for f in nc.m.functions:
        for bb in f.blocks:
            new = []
            for inst in bb.instructions:
                si = inst.sync_info
                if si is not None and len(si.on_wait) > max_waits:
                    waits = list(si.on_wait)
                    extra, keep = waits[:-max_waits], waits[-max_waits:]
                    for j in range(0, len(extra), max_waits):
                        d = bass_rust.InstNoOp(name=f"{inst.name}-w{j}",
                                               ins=[], outs=[])
                        d.engine = inst.engine
                        d.sync_info = mybir.SyncInfo(
                            on_wait=extra[j:j + max_waits], on_update=[])
                        new.append(d)
                    inst.sync_info = mybir.SyncInfo(
                        on_wait=keep, on_update=list(si.on_update))
                new.append(inst)
            bb.instructions = new


def _build_program():
    nc = bass.Bass("TRN2", target_bir_lowering=False, debug=False)

    x_t = nc.dram_tensor("x_t", [NB, B], F32R, kind="ExternalInput").ap()
    w1t = nc.dram_tensor("w1t", [P, NB * KC], F32, kind="ExternalInput").ap()
    b1t = nc.dram_tensor("b1t", [P, NB * KC], F32, kind="ExternalInput").ap()
    w2t = nc.dram_tensor("w2t", [NB, KC, P, EMB], F32R, kind="ExternalInput").ap()
    b2r = nc.dram_tensor("b2r", [NB, EMB], F32R, kind="ExternalInput").ap()
    ones_d = nc.dram_tensor("ones_d", [1, P], F32R, kind="ExternalInput").ap()
    out = nc.dram_tensor("out", [B, NB, EMB], F32, kind="ExternalOutput").ap()

    AF = mybir.ActivationFunctionType

    with tile.TileContext(nc) as tc:
        with (
            tc.tile_pool(name="consts", bufs=1) as consts,
            tc.tile_pool(name="xp", bufs=2) as xp,
            tc.tile_pool(name="w2p", bufs=2) as w2p,
            tc.tile_pool(name="b2p", bufs=2) as b2p,
            tc.tile_pool(name="hp", bufs=3) as hp,
            tc.tile_pool(name="op", bufs=5) as op,
        ):
            def load_w2(i):
                w2s = w2p.tile([P, KC, EMB], F32R, tag="w2s", name="w2s")
                nc.sync.dma_start(w2s[:], w2t[i].rearrange("k p e -> p k e"))
                return w2s

            # branch 0's W2 is on the critical path - start it first
            w2s0 = load_w2(0)

            w1s = consts.tile([P, NB * KC], F32, name="w1s")
            nc.sync.dma_start(w1s[:], w1t[:])
            b1s = consts.tile([P, NB * KC], F32, name="b1s")
            nc.sync.dma_start(b1s[:], b1t[:])
            # all-ones stationary operand for partition-broadcast matmuls
            ones = consts.tile([1, P], F32R, name="ones")
            nc.sync.dma_start(ones[:], ones_d[:])
            # x and b2 broadcast across partitions for ALL branches (only
            # 36 KB/partition each) - built once in the prologue
            xall = consts.tile([P, NB, B], F32, name="xall")
            b2all = consts.tile([P, NB, EMB], F32, name="b2all")

            # ---- prologue: partition-broadcast x and b2 via ones-matmul ----
            with tc.tile_pool(name="bcp", bufs=2, space="PSUM") as bcp:
                for i in range(NB):
                    xrow = xp.tile([1, B], F32R, tag="xrow", name="xrow")
                    nc.sync.dma_start(xrow[:], x_t[i:i + 1, :])
                    xps = bcp.tile([P, B], F32, tag="bcps", name="xps")
                    for n in range(B // 512):
                        nc.tensor.matmul(xps[:, n * 512:(n + 1) * 512],
                                         ones[:], xrow[:, n * 512:(n + 1) * 512],
                                         start=True, stop=True)
                    nc.vector.tensor_copy(xall[:, i, :], xps[:])
                for i in range(NB):
                    b2row = b2p.tile([1, EMB], F32R, tag="b2row", name="b2row")
                    nc.sync.dma_start(b2row[:], b2r[i:i + 1, :])
                    b2ps = bcp.tile([P, EMB], F32, tag="bcps", name="b2ps")
                    for n in range(EMB // 512):
                        nc.tensor.matmul(b2ps[:, n * 512:(n + 1) * 512],
                                         ones[:], b2row[:, n * 512:(n + 1) * 512],
                                         start=True, stop=True)
                    nc.vector.tensor_copy(b2all[:, i, :], b2ps[:])

            with tc.tile_pool(name="psp", bufs=4, space="PSUM") as psp:
                def prep_branch(i, w2s=None):
                    """W2 DMA + fc1 for branch i, issued one branch ahead so
                    the ACT chain overlaps the previous branch's GEMM."""
                    if w2s is None:
                        w2s = load_w2(i)
                    # fc1: h^T[k] = Lrelu(x_bcast * W1[chunk k] + b1), [128, B]
                    ht = hp.tile([P, KC, B], F32R, tag="ht", name="ht")
                    for k in range(KC):
                        c = i * KC + k
                        nc.scalar.activation(
                            ht[:, k, :], xall[:, i, :], AF.Lrelu,
                            bias=b1s[:, c:c + 1], scale=w1s[:, c:c + 1],
                            alpha=NEG_SLOPE,
                        )
                    return ht, w2s

                def gemm_branch(i, ht, w2s):
                    for m in range(MC):
                        ps0 = psp.tile([P, 512], F32, tag="ps0", name="ps0")
                        ps1 = psp.tile([P, 512], F32, tag="ps1", name="ps1")
                        for k in range(KC):
                            lhsT = ht[:, k, m * P:(m + 1) * P]
                            nc.tensor.matmul(ps0[:], lhsT, w2s[:, k, 0:512],
                                             start=(k == 0), stop=(k == KC - 1))
                            nc.tensor.matmul(ps1[:], lhsT, w2s[:, k, 512:1024],
                                             start=(k == 0), stop=(k == KC - 1))
                        osb = op.tile([P, EMB], F32, tag="osb", name="osb")
                        nc.vector.tensor_add(osb[:, 0:512], ps0[:],
                                             b2all[:, i, 0:512])
                        nc.vector.tensor_add(osb[:, 512:1024], ps1[:],
                                             b2all[:, i, 512:1024])
                        nc.sync.dma_start(out[m * P:(m + 1) * P, i, :], osb[:])

                pending = prep_branch(0, w2s0)
                for i in range(NB):
                    nxt = prep_branch(i + 1) if i + 1 < NB else None
                    gemm_branch(i, *pending)
                    pending = nxt

    _split_excess_waits(nc)
    return nc


def _get_program():
    global _compiled
    if _compiled is None:
        _compiled = _build_program()
    return _compiled


def _shard_inputs(x, W1, b1, W2, b2):
    """Build the 8 per-core input maps."""
    in_maps = []
    for c in range(N_CORES):
        half = c // BRANCH_SPLIT
        grp = c % BRANCH_SPLIT
        rows = slice(half * B, (half + 1) * B)
        brs = slice(grp * NB, (grp + 1) * NB)

        x_t = np.ascontiguousarray(x[rows, brs].T)        # [NB, B]

        w1g = W1[brs].reshape(NB, KC, P)                  # [NB, KC, 128]
        w1t = np.ascontiguousarray(w1g.transpose(2, 0, 1).reshape(P, NB * KC))
        b1g = b1[brs].reshape(NB, KC, P)
        b1t = np.ascontiguousarray(b1g.transpose(2, 0, 1).reshape(P, NB * KC))

        w2t = np.ascontiguousarray(W2[brs].reshape(NB, KC, P, EMB))
        b2r = np.ascontiguousarray(b2[brs])               # [NB, EMB]

        in_maps.append({"x_t": x_t, "w1t": w1t, "b1t": b1t,
                        "w2t": w2t, "b2r": b2r,
                        "ones_d": np.ones((1, P), dtype=np.float32)})
    return in_maps


def kernel(x, W1, b1, W2, b2, _trace=False):
    x = np.asarray(x, dtype=np.float32)
    W1 = np.asarray(W1, dtype=np.float32)
    b1 = np.asarray(b1, dtype=np.float32)
    W2 = np.asarray(W2, dtype=np.float32)
    b2 = np.asarray(b2, dtype=np.float32)

    nc = _get_program()
    in_maps = _shard_inputs(x, W1, b1, W2, b2)
    res = run_bass_kernel_spmd(nc, in_maps, list(range(N_CORES)), trace=_trace)

    out = np.empty((B_FULL, IN_DIM, EMB), dtype=np.float32)
    for c in range(N_CORES):
        half = c // BRANCH_SPLIT
        grp = c % BRANCH_SPLIT
        out[half * B:(half + 1) * B, grp * NB:(grp + 1) * NB, :] = \
            res.results[c]["out"]

    if _trace:
        kernel.last_exec_time_ns = res.exec_time_ns
    return out


kernel.last_exec_time_ns = None


# revision 16
# speedup vs baseline: 1.0873x; 1.0873x over previous
"""Trainium2 Bass kernel for nn_CEmbedder_L: 36 independent scalar-input MLPs.

Reference computation (fp32):
    h   = leaky_relu(x[:, :, None] * W1[None] + b1[None])   # [B, 36, 512]
    out = einsum('bih,ihd->bid', h, W2) + b2[None]          # [B, 36, 1024]

Sharding across 8 NeuronCores, perfectly balanced with minimal W2
duplication: core c owns branches [4c, 4c+4) for the FULL batch (2048)
plus ONE half-batch (1024 rows) share of branch 32 + c%4 (batch half
c//4). Every core therefore runs the identical program on 4 full branch
slots + 1 half slot: same FLOPs, same bytes. Branches 0-31 load W2 once
chip-wide; only branches 32-35 are loaded twice.

Per-core dataflow per branch slot:
  - x column is replicated across partitions once in a prologue via a
    ones-matmul on the PE (K=1 outer product into PSUM, copied to SBUF).
  - fc1 per 128-wide hid chunk k is one ScalarE activation:
    h^T[k] = Lrelu(x_bcast * W1[k-chunk] + b1[k-chunk]) with
    per-partition scale/bias - h^T laid out [hid, batch] as the matmul
    needs. Output dtype float32r (rounded fp32, full-rate on the PE,
    ~2e-4 relative rounding vs 4x-slower exact fp32).
  - fc2 is a [batch, 512] @ [512, 1024] GEMM on TensorE with float32r
    operands: out[m, n] += h^T[k][:, m-chunk].T @ W2[k][:, n-chunk]
    accumulated over k in fp32 PSUM; VectorE adds b2 (broadcast in the
    prologue) while evacuating PSUM -> SBUF; DMA writes [128, 1024]
    rows out.
"""

import sys

if "/opt/trn_rl_repo" not in sys.path:
    sys.path.insert(0, "/opt/trn_rl_repo")

import numpy as np

import concourse.bass as bass
import concourse.mybir as mybir
import concourse.tile as tile
from concourse.bass_utils import run_bass_kernel_spmd

B_FULL = 2048
IN_DIM = 36
HID = 512
EMB = 1024
NEG_SLOPE = 0.01

N_CORES = 8
NBF = 4                    # full-batch branches per core
NSLOT = NBF + 1            # + one half-batch slot
B0 = B_FULL                # full slot batch
B1 = B_FULL // 2           # half slot batch
KC = HID // 128            # 4 contraction chunks of 128
P = 128

F32 = mybir.dt.float32
F32R = mybir.dt.float32r

_compiled = None


def _split_excess_waits(nc, max_waits=1):
    """The walrus build in this container rejects instructions carrying
    more than one sync wait ("Too many sync wait commands", setupSyncWait)
    instead of auto-splitting them. Move excess waits onto same-engine
    NoOp carriers placed immediately before the instruction -
    engine-serial execution preserves wait-then-proceed semantics."""
    import bass_rust
    for f in nc.m.functions:
        for bb in f.blocks:
            new = []
            for inst in bb.instructions:
                si = inst.sync_info
                if si is not None and len(si.on_wait) > max_waits:
                    waits = list(si.on_wait)
                    extra, keep = waits[:-max_waits], waits[-max_waits:]
                    for j in range(0, len(extra), max_waits):
                        d = bass_rust.InstNoOp(name=f"{inst.name}-w{j}",
                                               ins=[], outs=[])
                        d.engine = inst.engine
                        d.sync_info = mybir.SyncInfo(
                            on_wait=extra[j:j + max_waits], on_update=[])
                        new.append(d)
                    inst.sync_info = mybir.SyncInfo(
                        on_wait=keep, on_update=list(si.on_update))
                new.append(inst)
            bb.instructions = new


def _build_program():
    nc = bass.Bass("TRN2", target_bir_lowering=False, debug=False)

    x_tf = nc.dram_tensor("x_tf", [NBF, B0], F32R, kind="ExternalInput").ap()
    x_th = nc.dram_tensor("x_th", [1, B1], F32R, kind="ExternalInput").ap()
    w1t = nc.dram_tensor("w1t", [P, NSLOT * KC], F32, kind="ExternalInput").ap()
    b1t = nc.dram_tensor("b1t", [P, NSLOT * KC], F32, kind="ExternalInput").ap()
    w2t = nc.dram_tensor("w2t", [NSLOT, KC, P, EMB], F32R,
                         kind="ExternalInput").ap()
    b2r = nc.dram_tensor("b2r", [NSLOT, EMB], F32R, kind="ExternalInput").ap()
    ones_d = nc.dram_tensor("ones_d", [1, P], F32R, kind="ExternalInput").ap()
    outf = nc.dram_tensor("outf", [B0, NBF, EMB], F32,
                          kind="ExternalOutput").ap()
    outh = nc.dram_tensor("outh", [B1, EMB], F32, kind="ExternalOutput").ap()

    AF = mybir.ActivationFunctionType

    with tile.TileContext(nc) as tc:
        with (
            tc.tile_pool(name="consts", bufs=1) as consts,
            tc.tile_pool(name="xp", bufs=1) as xp,
            tc.tile_pool(name="w2p", bufs=2) as w2p,
            tc.tile_pool(name="b2p", bufs=2) as b2p,
            tc.tile_pool(name="hp", bufs=2) as hp,
            tc.tile_pool(name="op", bufs=4) as op,
        ):
            def load_w2(s):
                w2s = w2p.tile([P, KC, EMB], F32R, tag="w2s", name="w2s")
                nc.sync.dma_start(w2s[:], w2t[s].rearrange("k p e -> p k e"))
                return w2s

            # branch slot 0's W2 is on the critical path - start it first
            w2s0 = load_w2(0)

            w1s = consts.tile([P, NSLOT * KC], F32, name="w1s")
            nc.sync.dma_start(w1s[:], w1t[:])
            b1s = consts.tile([P, NSLOT * KC], F32, name="b1s")
            nc.sync.dma_start(b1s[:], b1t[:])
            # all-ones stationary operand for partition-broadcast matmuls
            ones = consts.tile([1, P], F32R, name="ones")
            nc.sync.dma_start(ones[:], ones_d[:])
            # x and b2 broadcast across partitions for ALL slots (small)
            xall_f = consts.tile([P, NBF, B0], F32, name="xall_f")
            xall_h = consts.tile([P, B1], F32, name="xall_h")
            b2all = consts.tile([P, NSLOT, EMB], F32, name="b2all")

            # ---- prologue: partition-broadcast x and b2 via ones-matmul ----
            with tc.tile_pool(name="bcp", bufs=2, space="PSUM") as bcp:
                def bcast(dst, src_row, width):
                    """dst [P, width] <- broadcast of src_row [1, width]."""
                    ps = bcp.tile([P, 1024], F32, tag="bcps", name="ps")
                    for n in range(width // 512):
                        nc.tensor.matmul(ps[:, n * 512:(n + 1) * 512], ones[:],
                                         src_row[:, n * 512:(n + 1) * 512],
                                         start=True, stop=True)
                    nc.vector.tensor_copy(dst[:], ps[:, 0:width])

                for s in range(NBF):
                    xrow = xp.tile([1, B0], F32R, tag="xrow", name="xrow")
                    nc.sync.dma_start(xrow[:], x_tf[s:s + 1, :])
                    for h in range(B0 // 1024):
                        bcast(xall_f[:, s, h * 1024:(h + 1) * 1024],
                              xrow[:, h * 1024:(h + 1) * 1024], 1024)
                xrowh = xp.tile([1, B1], F32R, tag="xrowh", name="xrowh")
                nc.sync.dma_start(xrowh[:], x_th[0:1, :])
                bcast(xall_h[:, :], xrowh[:, :], B1)
                for s in range(NSLOT):
                    b2row = b2p.tile([1, EMB], F32R, tag="b2row", name="b2row")
                    nc.sync.dma_start(b2row[:], b2r[s:s + 1, :])
                    bcast(b2all[:, s, :], b2row[:, :], EMB)

            with tc.tile_pool(name="psp", bufs=4, space="PSUM") as psp:
                def slot_cfg(s):
                    if s < NBF:
                        return B0, xall_f[:, s, :]
                    return B1, xall_h[:, :]

                def prep_slot(s, w2s=None):
                    """W2 DMA + fc1 for slot s, issued one slot ahead so the
                    ACT chain overlaps the previous slot's GEMM."""
                    Bs, xsrc = slot_cfg(s)
                    if w2s is None:
                        w2s = load_w2(s)
                    # fc1: h^T[k] = Lrelu(x_bcast * W1[chunk k] + b1)
                    ht = hp.tile([P, KC, Bs], F32R, tag="ht", name="ht",
                                 padded_shape=[P, KC, B0])
                    for k in range(KC):
                        c = s * KC + k
                        nc.scalar.activation(
                            ht[:, k, :], xsrc, AF.Lrelu,
                            bias=b1s[:, c:c + 1], scale=w1s[:, c:c + 1],
                            alpha=NEG_SLOPE,
                        )
                    return ht, w2s

                def gemm_slot(s, ht, w2s):
                    Bs, _ = slot_cfg(s)
                    for m in range(Bs // P):
                        ps0 = psp.tile([P, 512], F32, tag="ps0", name="ps0")
                        ps1 = psp.tile([P, 512], F32, tag="ps1", name="ps1")
                        for k in range(KC):
                            lhsT = ht[:, k, m * P:(m + 1) * P]
                            nc.tensor.matmul(ps0[:], lhsT, w2s[:, k, 0:512],
                                             start=(k == 0), stop=(k == KC - 1))
                            nc.tensor.matmul(ps1[:], lhsT, w2s[:, k, 512:1024],
                                             start=(k == 0), stop=(k == KC - 1))
                        osb = op.tile([P, EMB], F32, tag="osb", name="osb")
                        nc.vector.tensor_add(osb[:, 0:512], ps0[:],
                                             b2all[:, s, 0:512])
                        nc.vector.tensor_add(osb[:, 512:1024], ps1[:],
                                             b2all[:, s, 512:1024])
                        if s < NBF:
                            nc.sync.dma_start(outf[m * P:(m + 1) * P, s, :],
                                              osb[:])
                        else:
                            nc.sync.dma_start(outh[m * P:(m + 1) * P, :],
                                              osb[:])

                pending = prep_slot(0, w2s0)
                for s in range(NSLOT):
                    nxt = prep_slot(s + 1) if s + 1 < NSLOT else None
                    gemm_slot(s, *pending)
                    pending = nxt

    _split_excess_waits(nc)
    return nc


def _get_program():
    global _compiled
    if _compiled is None:
        _compiled = _build_program()
    return _compiled


def _shard_inputs(x, W1, b1, W2, b2):
    """Build the 8 per-core input maps. Core c: full branches [4c, 4c+4),
    half slot = branch 32 + c%4, batch half c//4."""
    in_maps = []
    ones = np.ones((1, P), dtype=np.float32)
    for c in range(N_CORES):
        fb = list(range(4 * c, 4 * c + 4))
        hb = 32 + (c % 4)
        half = c // 4
        hrows = slice(half * B1, (half + 1) * B1)
        slots = fb + [hb]

        x_tf = np.ascontiguousarray(x[:, fb].T)              # [4, 2048]
        x_th = np.ascontiguousarray(x[hrows, hb][None, :])   # [1, 1024]

        w1g = W1[slots].reshape(NSLOT, KC, P)
        w1t = np.ascontiguousarray(
            w1g.transpose(2, 0, 1).reshape(P, NSLOT * KC))
        b1g = b1[slots].reshape(NSLOT, KC, P)
        b1t = np.ascontiguousarray(
            b1g.transpose(2, 0, 1).reshape(P, NSLOT * KC))

        w2t = np.ascontiguousarray(W2[slots].reshape(NSLOT, KC, P, EMB))
        b2r = np.ascontiguousarray(b2[slots])                # [5, EMB]

        in_maps.append({"x_tf": x_tf, "x_th": x_th, "w1t": w1t, "b1t": b1t,
                        "w2t": w2t, "b2r": b2r, "ones_d": ones})
    return in_maps


def kernel(x, W1, b1, W2, b2, _trace=False):
    x = np.asarray(x, dtype=np.float32)
    W1 = np.asarray(W1, dtype=np.float32)
    b1 = np.asarray(b1, dtype=np.float32)
    W2 = np.asarray(W2, dtype=np.float32)
    b2 = np.asarray(b2, dtype=np.float32)

    nc = _get_program()
    in_maps = _shard_inputs(x, W1, b1, W2, b2)
    res = run_bass_kernel_spmd(nc, in_maps, list(range(N_CORES)), trace=_trace)

    out = np.empty((B_FULL, IN_DIM, EMB), dtype=np.float32)
    for c in range(N_CORES):
        fb = list(range(4 * c, 4 * c + 4))
        hb = 32 + (c % 4)
        half = c // 4
        out[:, fb, :] = res.results[c]["outf"].transpose(0, 1, 2)
        out[half * B1:(half + 1) * B1, hb, :] = res.results[c]["outh"]

    if _trace:
        kernel.last_exec_time_ns = res.exec_time_ns
    return out


kernel.last_exec_time_ns = None


# revision 18
# speedup vs baseline: 1.1435x; 1.0516x over previous
"""Trainium2 Bass kernel for nn_CEmbedder_L: 36 independent scalar-input MLPs.

Reference computation (fp32):
    h   = leaky_relu(x[:, :, None] * W1[None] + b1[None])   # [B, 36, 512]
    out = einsum('bih,ihd->bid', h, W2) + b2[None]          # [B, 36, 1024]

Sharding across 8 NeuronCores, perfectly balanced with minimal W2
duplication: core c owns branches [4c, 4c+4) for the FULL batch (2048)
plus ONE half-batch (1024 rows) share of branch 32 + c%4 (batch half
c//4). Every core therefore runs the identical program on 4 full branch
slots + 1 half slot: same FLOPs, same bytes. Branches 0-31 load W2 once
chip-wide; only branches 32-35 are loaded twice.

Per-core dataflow per branch slot:
  - x column is replicated across partitions once in a prologue via a
    ones-matmul on the PE (K=1 outer product into PSUM, copied to SBUF).
  - fc1 per 128-wide hid chunk k is one ScalarE activation:
    h^T[k] = Lrelu(x_bcast * W1[k-chunk] + b1[k-chunk]) with
    per-partition scale/bias - h^T laid out [hid, batch] as the matmul
    needs. Output dtype float32r (rounded fp32, full-rate on the PE,
    ~2e-4 relative rounding vs 4x-slower exact fp32).
  - fc2 is a [batch, 512] @ [512, 1024] GEMM on TensorE with float32r
    operands: out[m, n] += h^T[k][:, m-chunk].T @ W2[k][:, n-chunk]
    accumulated over k in fp32 PSUM; VectorE adds b2 (broadcast in the
    prologue) while evacuating PSUM -> SBUF; DMA writes [128, 1024]
    rows out.
"""

import sys

if "/opt/trn_rl_repo" not in sys.path:
    sys.path.insert(0, "/opt/trn_rl_repo")

import numpy as np

import concourse.bass as bass
import concourse.mybir as mybir
import concourse.tile as tile
from concourse.bass_utils import run_bass_kernel_spmd

B_FULL = 2048
IN_DIM = 36
HID = 512
EMB = 1024
NEG_SLOPE = 0.01

N_CORES = 8
NBF = 4                    # full-batch branches per core
NSLOT = NBF + 1            # + one half-batch slot
B0 = B_FULL                # full slot batch
B1 = B_FULL // 2           # half slot batch
KC = HID // 128            # 4 contraction chunks of 128
P = 128

F32 = mybir.dt.float32
F32R = mybir.dt.float32r

_compiled = None


def _split_excess_waits(nc, max_waits=1):
    """The walrus build in this container rejects instructions carrying
    more than one sync wait ("Too many sync wait commands", setupSyncWait)
    instead of auto-splitting them. Move excess waits onto same-engine
    NoOp carriers placed immediately before the instruction -
    engine-serial execution preserves wait-then-proceed semantics."""
    import bass_rust
    for f in nc.m.functions:
        for bb in f.blocks:
            new = []
            for inst in bb.instructions:
                si = inst.sync_info
                if si is not None and len(si.on_wait) > max_waits:
                    waits = list(si.on_wait)
                    extra, keep = waits[:-max_waits], waits[-max_waits:]
                    for j in range(0, len(extra), max_waits):
                        d = bass_rust.InstNoOp(name=f"{inst.name}-w{j}",
                                               ins=[], outs=[])
                        d.engine = inst.engine
                        d.sync_info = mybir.SyncInfo(
                            on_wait=extra[j:j + max_waits], on_update=[])
                        new.append(d)
                    inst.sync_info = mybir.SyncInfo(
                        on_wait=keep, on_update=list(si.on_update))
                new.append(inst)
            bb.instructions = new


def _build_program():
    nc = bass.Bass("TRN2", target_bir_lowering=False, debug=False)

    x_tf = nc.dram_tensor("x_tf", [NBF, B0], F32R, kind="ExternalInput").ap()
    x_th = nc.dram_tensor("x_th", [1, B1], F32R, kind="ExternalInput").ap()
    w1t = nc.dram_tensor("w1t", [P, NSLOT * KC], F32, kind="ExternalInput").ap()
    b1t = nc.dram_tensor("b1t", [P, NSLOT * KC], F32, kind="ExternalInput").ap()
    w2t = nc.dram_tensor("w2t", [NSLOT, KC, P, EMB], F32R,
                         kind="ExternalInput").ap()
    b2r = nc.dram_tensor("b2r", [NSLOT, EMB], F32R, kind="ExternalInput").ap()
    ones_d = nc.dram_tensor("ones_d", [1, P], F32R, kind="ExternalInput").ap()
    outf = nc.dram_tensor("outf", [B0, NBF, EMB], F32,
                          kind="ExternalOutput").ap()
    outh = nc.dram_tensor("outh", [B1, EMB], F32, kind="ExternalOutput").ap()

    AF = mybir.ActivationFunctionType

    with tile.TileContext(nc) as tc:
        with (
            tc.tile_pool(name="consts", bufs=1) as consts,
            tc.tile_pool(name="xp", bufs=2) as xp,
            tc.tile_pool(name="w2p", bufs=2) as w2p,
            tc.tile_pool(name="b2p", bufs=2) as b2p,
            tc.tile_pool(name="hp", bufs=2) as hp,
            tc.tile_pool(name="op", bufs=3) as op,
        ):
            def load_w2(s):
                w2s = w2p.tile([P, KC, EMB], F32R, tag="w2s", name="w2s")
                nc.sync.dma_start(w2s[:], w2t[s].rearrange("k p e -> p k e"))
                return w2s

            # Small control DMAs first: the broadcast/fc1 chain depends on
            # them, and a 2MB W2 transfer issued ahead would starve them
            # for ~15us on the cold DMA path.
            w1s = consts.tile([P, NSLOT * KC], F32, name="w1s")
            nc.sync.dma_start(w1s[:], w1t[:])
            b1s = consts.tile([P, NSLOT * KC], F32, name="b1s")
            nc.sync.dma_start(b1s[:], b1t[:])
            # all-ones stationary operand for partition-broadcast matmuls
            ones = consts.tile([1, P], F32R, name="ones")
            nc.sync.dma_start(ones[:], ones_d[:])
            # x and b2 broadcast across partitions for ALL slots (small)
            xall_f = consts.tile([P, NBF, B0], F32, name="xall_f")
            xall_h = consts.tile([P, B1], F32, name="xall_h")
            b2all = consts.tile([P, NSLOT, EMB], F32, name="b2all")

            # ---- prologue: partition-broadcast x and b2 via ones-matmul ----
            with tc.tile_pool(name="bcp", bufs=2, space="PSUM") as bcp:
                def bcast(dst, src_row, width):
                    """dst [P, width] <- broadcast of src_row [1, width]."""
                    ps = bcp.tile([P, 1024], F32, tag="bcps", name="ps")
                    for n in range(width // 512):
                        nc.tensor.matmul(ps[:, n * 512:(n + 1) * 512], ones[:],
                                         src_row[:, n * 512:(n + 1) * 512],
                                         start=True, stop=True)
                    nc.vector.tensor_copy(dst[:], ps[:, 0:width])

                xrow0 = xp.tile([1, B0], F32R, tag="xrow", name="xrow0")
                nc.sync.dma_start(xrow0[:], x_tf[0:1, :])
                xrowh = xp.tile([1, B1], F32R, tag="xrowh", name="xrowh",
                                bufs=1)
                nc.sync.dma_start(xrowh[:], x_th[0:1, :])

                # W2 for the first slot - streams under the prologue
                w2s0 = load_w2(0)

                for h in range(B0 // 1024):
                    bcast(xall_f[:, 0, h * 1024:(h + 1) * 1024],
                          xrow0[:, h * 1024:(h + 1) * 1024], 1024)
                bcast(xall_h[:, :], xrowh[:, :], B1)
                for s in range(1, NBF):
                    xrow = xp.tile([1, B0], F32R, tag="xrow", name="xrow")
                    nc.sync.dma_start(xrow[:], x_tf[s:s + 1, :])
                    for h in range(B0 // 1024):
                        bcast(xall_f[:, s, h * 1024:(h + 1) * 1024],
                              xrow[:, h * 1024:(h + 1) * 1024], 1024)
                for s in range(NSLOT):
                    b2row = b2p.tile([1, EMB], F32R, tag="b2row", name="b2row")
                    nc.sync.dma_start(b2row[:], b2r[s:s + 1, :])
                    bcast(b2all[:, s, :], b2row[:, :], EMB)

            with tc.tile_pool(name="psp", bufs=4, space="PSUM") as psp:
                def slot_cfg(s):
                    if s < NBF:
                        return B0, xall_f[:, s, :]
                    return B1, xall_h[:, :]

                def prep_slot(s, w2s=None):
                    """W2 DMA + fc1 for slot s, issued one slot ahead so the
                    ACT chain overlaps the previous slot's GEMM."""
                    Bs, xsrc = slot_cfg(s)
                    if w2s is None:
                        w2s = load_w2(s)
                    # fc1: h^T[k] = Lrelu(x_bcast * W1[chunk k] + b1)
                    ht = hp.tile([P, KC, Bs], F32R, tag="ht", name="ht",
                                 padded_shape=[P, KC, B0])
                    for k in range(KC):
                        c = s * KC + k
                        nc.scalar.activation(
                            ht[:, k, :], xsrc, AF.Lrelu,
                            bias=b1s[:, c:c + 1], scale=w1s[:, c:c + 1],
                            alpha=NEG_SLOPE,
                        )
                    return ht, w2s

                def gemm_slot(s, ht, w2s):
                    Bs, _ = slot_cfg(s)
                    for m in range(Bs // P):
                        ps0 = psp.tile([P, 512], F32, tag="ps0", name="ps0")
                        ps1 = psp.tile([P, 512], F32, tag="ps1", name="ps1")
                        for k in range(KC):
                            lhsT = ht[:, k, m * P:(m + 1) * P]
                            nc.tensor.matmul(ps0[:], lhsT, w2s[:, k, 0:512],
                                             start=(k == 0), stop=(k == KC - 1))
                            nc.tensor.matmul(ps1[:], lhsT, w2s[:, k, 512:1024],
                                             start=(k == 0), stop=(k == KC - 1))
                        osb = op.tile([P, EMB], F32, tag="osb", name="osb")
                        nc.vector.tensor_add(osb[:, 0:512], ps0[:],
                                             b2all[:, s, 0:512])
                        nc.vector.tensor_add(osb[:, 512:1024], ps1[:],
                                             b2all[:, s, 512:1024])
                        if s < NBF:
                            nc.sync.dma_start(outf[m * P:(m + 1) * P, s, :],
                                              osb[:])
                        else:
                            nc.sync.dma_start(outh[m * P:(m + 1) * P, :],
                                              osb[:])

                pending = prep_slot(0, w2s0)
                for s in range(NSLOT):
                    nxt = prep_slot(s + 1) if s + 1 < NSLOT else None
                    gemm_slot(s, *pending)
                    pending = nxt

    _split_excess_waits(nc)
    return nc


def _get_program():
    global _compiled
    if _compiled is None:
        _compiled = _build_program()
    return _compiled


def _shard_inputs(x, W1, b1, W2, b2):
    """Build the 8 per-core input maps. Core c: full branches [4c, 4c+4),
    half slot = branch 32 + c%4, batch half c//4."""
    in_maps = []
    ones = np.ones((1, P), dtype=np.float32)
    for c in range(N_CORES):
        fb = list(range(4 * c, 4 * c + 4))
        hb = 32 + (c % 4)
        half = c // 4
        hrows = slice(half * B1, (half + 1) * B1)
        slots = fb + [hb]

        x_tf = np.ascontiguousarray(x[:, fb].T)              # [4, 2048]
        x_th = np.ascontiguousarray(x[hrows, hb][None, :])   # [1, 1024]

        w1g = W1[slots].reshape(NSLOT, KC, P)
        w1t = np.ascontiguousarray(
            w1g.transpose(2, 0, 1).reshape(P, NSLOT * KC))
        b1g = b1[slots].reshape(NSLOT, KC, P)
        b1t = np.ascontiguousarray(
            b1g.transpose(2, 0, 1).reshape(P, NSLOT * KC))

        w2t = np.ascontiguousarray(W2[slots].reshape(NSLOT, KC, P, EMB))
        b2r = np.ascontiguousarray(b2[slots])                # [5, EMB]

        in_maps.append({"x_tf": x_tf, "x_th": x_th, "w1t": w1t, "b1t": b1t,
                        "w2t": w2t, "b2r": b2r, "ones_d": ones})
    return in_maps


def kernel(x, W1, b1, W2, b2, _trace=False):
    x = np.asarray(x, dtype=np.float32)
    W1 = np.asarray(W1, dtype=np.float32)
    b1 = np.asarray(b1, dtype=np.float32)
    W2 = np.asarray(W2, dtype=np.float32)
    b2 = np.asarray(b2, dtype=np.float32)

    nc = _get_program()
    in_maps = _shard_inputs(x, W1, b1, W2, b2)
    res = run_bass_kernel_spmd(nc, in_maps, list(range(N_CORES)), trace=_trace)

    out = np.empty((B_FULL, IN_DIM, EMB), dtype=np.float32)
    for c in range(N_CORES):
        fb = list(range(4 * c, 4 * c + 4))
        hb = 32 + (c % 4)
        half = c // 4
        out[:, fb, :] = res.results[c]["outf"].transpose(0, 1, 2)
        out[half * B1:(half + 1) * B1, hb, :] = res.results[c]["outh"]

    if _trace:
        kernel.last_exec_time_ns = res.exec_time_ns
    return out


kernel.last_exec_time_ns = None


# revision 19
# speedup vs baseline: 1.2121x; 1.0600x over previous
"""Trainium2 Bass kernel for nn_CEmbedder_L: 36 independent scalar-input MLPs.

Reference computation (fp32):
    h   = leaky_relu(x[:, :, None] * W1[None] + b1[None])   # [B, 36, 512]
    out = einsum('bih,ihd->bid', h, W2) + b2[None]          # [B, 36, 1024]

Sharding across 8 NeuronCores, perfectly balanced with minimal W2
duplication: core c owns branches [4c, 4c+4) for the FULL batch (2048)
plus ONE half-batch (1024 rows) share of branch 32 + c%4 (batch half
c//4). Every core therefore runs the identical program on 4 full branch
slots + 1 half slot: same FLOPs, same bytes. Branches 0-31 load W2 once
chip-wide; only branches 32-35 are loaded twice.

Per-core dataflow per branch slot:
  - x column is replicated across partitions once in a prologue via a
    ones-matmul on the PE (K=1 outer product into PSUM, copied to SBUF).
  - fc1 per 128-wide hid chunk k is one ScalarE activation:
    h^T[k] = Lrelu(x_bcast * W1[k-chunk] + b1[k-chunk]) with
    per-partition scale/bias - h^T laid out [hid, batch] as the matmul
    needs. Output dtype float32r (rounded fp32, full-rate on the PE,
    ~2e-4 relative rounding vs 4x-slower exact fp32).
  - fc2 is a [batch, 512] @ [512, 1024] GEMM on TensorE with float32r
    operands: out[m, n] += h^T[k][:, m-chunk].T @ W2[k][:, n-chunk]
    accumulated over k in fp32 PSUM; VectorE adds b2 (broadcast in the
    prologue) while evacuating PSUM -> SBUF; DMA writes [128, 1024]
    rows out.
"""

import sys

if "/opt/trn_rl_repo" not in sys.path:
    sys.path.insert(0, "/opt/trn_rl_repo")

import numpy as np

import concourse.bass as bass
import concourse.mybir as mybir
import concourse.tile as tile
from concourse.bass_utils import run_bass_kernel_spmd

B_FULL = 2048
IN_DIM = 36
HID = 512
EMB = 1024
NEG_SLOPE = 0.01

N_CORES = 8
NBF = 4                    # full-batch branches per core
NSLOT = NBF + 1            # + one half-batch slot
B0 = B_FULL                # full slot batch
B1 = B_FULL // 2           # half slot batch
KC = HID // 128            # 4 contraction chunks of 128
P = 128

F32 = mybir.dt.float32
F32R = mybir.dt.float32r

_compiled = None


def _split_excess_waits(nc, max_waits=1):
    """The walrus build in this container rejects instructions carrying
    more than one sync wait ("Too many sync wait commands", setupSyncWait)
    instead of auto-splitting them. Move excess waits onto same-engine
    NoOp carriers placed immediately before the instruction -
    engine-serial execution preserves wait-then-proceed semantics."""
    import bass_rust
    for f in nc.m.functions:
        for bb in f.blocks:
            new = []
            for inst in bb.instructions:
                si = inst.sync_info
                if si is not None and len(si.on_wait) > max_waits:
                    waits = list(si.on_wait)
                    extra, keep = waits[:-max_waits], waits[-max_waits:]
                    for j in range(0, len(extra), max_waits):
                        d = bass_rust.InstNoOp(name=f"{inst.name}-w{j}",
                                               ins=[], outs=[])
                        d.engine = inst.engine
                        d.sync_info = mybir.SyncInfo(
                            on_wait=extra[j:j + max_waits], on_update=[])
                        new.append(d)
                    inst.sync_info = mybir.SyncInfo(
                        on_wait=keep, on_update=list(si.on_update))
                new.append(inst)
            bb.instructions = new


def _build_program():
    nc = bass.Bass("TRN2", target_bir_lowering=False, debug=False)

    x_tf = nc.dram_tensor("x_tf", [NBF, B0], F32R, kind="ExternalInput").ap()
    x_th = nc.dram_tensor("x_th", [1, B1], F32R, kind="ExternalInput").ap()
    w1t = nc.dram_tensor("w1t", [P, NSLOT * KC], F32, kind="ExternalInput").ap()
    b1t = nc.dram_tensor("b1t", [P, NSLOT * KC], F32, kind="ExternalInput").ap()
    w2t = nc.dram_tensor("w2t", [NSLOT, KC, P, EMB], F32R,
                         kind="ExternalInput").ap()
    b2r = nc.dram_tensor("b2r", [NSLOT, EMB], F32R, kind="ExternalInput").ap()
    ones_d = nc.dram_tensor("ones_d", [1, P], F32R, kind="ExternalInput").ap()
    outf = nc.dram_tensor("outf", [B0, NBF, EMB], F32,
                          kind="ExternalOutput").ap()
    outh = nc.dram_tensor("outh", [B1, EMB], F32, kind="ExternalOutput").ap()

    AF = mybir.ActivationFunctionType

    with tile.TileContext(nc) as tc:
        with (
            tc.tile_pool(name="consts", bufs=1) as consts,
            tc.tile_pool(name="xp", bufs=2) as xp,
            tc.tile_pool(name="w2p", bufs=2) as w2p,
            tc.tile_pool(name="b2p", bufs=2) as b2p,
            tc.tile_pool(name="hp", bufs=2) as hp,
            tc.tile_pool(name="op", bufs=3) as op,
        ):
            def load_w2(s):
                # one DMA per k-chunk: 4 parallel 512KB streams pipeline
                # better than one 2MB gather, and Tile's subtile deps let
                # early matmuls start before the whole tile lands
                w2s = w2p.tile([P, KC, EMB], F32R, tag="w2s", name="w2s")
                for k in range(KC):
                    nc.sync.dma_start(w2s[:, k, :], w2t[s, k])
                return w2s

            # Small control DMAs first: the broadcast/fc1 chain depends on
            # them, and a 2MB W2 transfer issued ahead would starve them
            # for ~15us on the cold DMA path.
            w1s = consts.tile([P, NSLOT * KC], F32, name="w1s")
            nc.sync.dma_start(w1s[:], w1t[:])
            b1s = consts.tile([P, NSLOT * KC], F32, name="b1s")
            nc.sync.dma_start(b1s[:], b1t[:])
            # all-ones stationary operand for partition-broadcast matmuls
            ones = consts.tile([1, P], F32R, name="ones")
            nc.sync.dma_start(ones[:], ones_d[:])
            # x and b2 broadcast across partitions for ALL slots (small)
            xall_f = consts.tile([P, NBF, B0], F32, name="xall_f")
            xall_h = consts.tile([P, B1], F32, name="xall_h")
            b2all = consts.tile([P, NSLOT, EMB], F32, name="b2all")

            # ---- prologue: partition-broadcast x and b2 via ones-matmul ----
            with tc.tile_pool(name="bcp", bufs=2, space="PSUM") as bcp:
                def bcast(dst, src_row, width):
                    """dst [P, width] <- broadcast of src_row [1, width]."""
                    ps = bcp.tile([P, 1024], F32, tag="bcps", name="ps")
                    for n in range(width // 512):
                        nc.tensor.matmul(ps[:, n * 512:(n + 1) * 512], ones[:],
                                         src_row[:, n * 512:(n + 1) * 512],
                                         start=True, stop=True)
                    nc.vector.tensor_copy(dst[:], ps[:, 0:width])

                xrow0 = xp.tile([1, B0], F32R, tag="xrow", name="xrow0")
                nc.sync.dma_start(xrow0[:], x_tf[0:1, :])
                xrowh = xp.tile([1, B1], F32R, tag="xrowh", name="xrowh",
                                bufs=1)
                nc.sync.dma_start(xrowh[:], x_th[0:1, :])

                # W2 for the first slot - streams under the prologue
                w2s0 = load_w2(0)

                for h in range(B0 // 1024):
                    bcast(xall_f[:, 0, h * 1024:(h + 1) * 1024],
                          xrow0[:, h * 1024:(h + 1) * 1024], 1024)
                bcast(xall_h[:, :], xrowh[:, :], B1)
                for s in range(1, NBF):
                    xrow = xp.tile([1, B0], F32R, tag="xrow", name="xrow")
                    nc.sync.dma_start(xrow[:], x_tf[s:s + 1, :])
                    for h in range(B0 // 1024):
                        bcast(xall_f[:, s, h * 1024:(h + 1) * 1024],
                              xrow[:, h * 1024:(h + 1) * 1024], 1024)
                for s in range(NSLOT):
                    b2row = b2p.tile([1, EMB], F32R, tag="b2row", name="b2row")
                    nc.sync.dma_start(b2row[:], b2r[s:s + 1, :])
                    bcast(b2all[:, s, :], b2row[:, :], EMB)

            with tc.tile_pool(name="psp", bufs=4, space="PSUM") as psp:
                def slot_cfg(s):
                    if s < NBF:
                        return B0, xall_f[:, s, :]
                    return B1, xall_h[:, :]

                def prep_slot(s, w2s=None):
                    """W2 DMA + fc1 for slot s, issued one slot ahead so the
                    ACT chain overlaps the previous slot's GEMM."""
                    Bs, xsrc = slot_cfg(s)
                    if w2s is None:
                        w2s = load_w2(s)
                    # fc1: h^T[k] = Lrelu(x_bcast * W1[chunk k] + b1)
                    ht = hp.tile([P, KC, Bs], F32R, tag="ht", name="ht",
                                 padded_shape=[P, KC, B0])
                    for k in range(KC):
                        c = s * KC + k
                        nc.scalar.activation(
                            ht[:, k, :], xsrc, AF.Lrelu,
                            bias=b1s[:, c:c + 1], scale=w1s[:, c:c + 1],
                            alpha=NEG_SLOPE,
                        )
                    return ht, w2s

                def gemm_slot(s, ht, w2s):
                    Bs, _ = slot_cfg(s)
                    for m in range(Bs // P):
                        ps0 = psp.tile([P, 512], F32, tag="ps0", name="ps0")
                        ps1 = psp.tile([P, 512], F32, tag="ps1", name="ps1")
                        for k in range(KC):
                            lhsT = ht[:, k, m * P:(m + 1) * P]
                            nc.tensor.matmul(ps0[:], lhsT, w2s[:, k, 0:512],
                                             start=(k == 0), stop=(k == KC - 1))
                            nc.tensor.matmul(ps1[:], lhsT, w2s[:, k, 512:1024],
                                             start=(k == 0), stop=(k == KC - 1))
                        osb = op.tile([P, EMB], F32, tag="osb", name="osb")
                        nc.vector.tensor_add(osb[:, 0:512], ps0[:],
                                             b2all[:, s, 0:512])
                        nc.vector.tensor_add(osb[:, 512:1024], ps1[:],
                                             b2all[:, s, 512:1024])
                        if s < NBF:
                            nc.sync.dma_start(outf[m * P:(m + 1) * P, s, :],
                                              osb[:])
                        else:
                            nc.sync.dma_start(outh[m * P:(m + 1) * P, :],
                                              osb[:])

                pending = prep_slot(0, w2s0)
                for s in range(NSLOT):
                    nxt = prep_slot(s + 1) if s + 1 < NSLOT else None
                    gemm_slot(s, *pending)
                    pending = nxt

    _split_excess_waits(nc)
    return nc


def _get_program():
    global _compiled
    if _compiled is None:
        _compiled = _build_program()
    return _compiled


def _shard_inputs(x, W1, b1, W2, b2):
    """Build the 8 per-core input maps. Core c: full branches [4c, 4c+4),
    half slot = branch 32 + c%4, batch half c//4."""
    in_maps = []
    ones = np.ones((1, P), dtype=np.float32)
    for c in range(N_CORES):
        fb = list(range(4 * c, 4 * c + 4))
        hb = 32 + (c % 4)
        half = c // 4
        hrows = slice(half * B1, (half + 1) * B1)
        slots = fb + [hb]

        x_tf = np.ascontiguousarray(x[:, fb].T)              # [4, 2048]
        x_th = np.ascontiguousarray(x[hrows, hb][None, :])   # [1, 1024]

        w1g = W1[slots].reshape(NSLOT, KC, P)
        w1t = np.ascontiguousarray(
            w1g.transpose(2, 0, 1).reshape(P, NSLOT * KC))
        b1g = b1[slots].reshape(NSLOT, KC, P)
        b1t = np.ascontiguousarray(
            b1g.transpose(2, 0, 1).reshape(P, NSLOT * KC))

        w2t = np.ascontiguousarray(W2[slots].reshape(NSLOT, KC, P, EMB))
        b2r = np.ascontiguousarray(b2[slots])                # [5, EMB]

        in_maps.append({"x_tf": x_tf, "x_th": x_th, "w1t": w1t, "b1t": b1t,
                        "w2t": w2t, "b2r": b2r, "ones_d": ones})
    return in_maps


def kernel(x, W1, b1, W2, b2, _trace=False):
    x = np.asarray(x, dtype=np.float32)
    W1 = np.asarray(W1, dtype=np.float32)
    b1 = np.asarray(b1, dtype=np.float32)
    W2 = np.asarray(W2, dtype=np.float32)
    b2 = np.asarray(b2, dtype=np.float32)

    nc = _get_program()
    in_maps = _shard_inputs(x, W1, b1, W2, b2)
    res = run_bass_kernel_spmd(nc, in_maps, list(range(N_CORES)), trace=_trace)

    out = np.empty((B_FULL, IN_DIM, EMB), dtype=np.float32)
    for c in range(N_CORES):
        fb = list(range(4 * c, 4 * c + 4))
        hb = 32 + (c % 4)
        half = c // 4
        out[:, fb, :] = res.results[c]["outf"].transpose(0, 1, 2)
        out[half * B1:(half + 1) * B1, hb, :] = res.results[c]["outh"]

    if _trace:
        kernel.last_exec_time_ns = res.exec_time_ns
    return out


kernel.last_exec_time_ns = None
